# revision 19
# baseline (speedup 1.0000x reference)
"""GATReasoner Trainium2 kernel: 2-layer GAT + reasoner MLP on 8 NeuronCores.

Strategy (dst-major, degree-balanced):
  - Host: add self-loops, sort nodes by in-degree, deal round-robin to the 8
    cores (near-perfect edge balance, uniform per-window max degree).  Edges
    are laid out dst-major: for each local destination d (one SBUF partition)
    the sources of its incoming edges occupy "j-slots" along the free dim.
  - Phase 0 (per core): h1ext = x @ [W1 | a_src-fold | a_dst-fold] for the
    core's nodes (bf16), AllGather #1 -> every core holds all nodes' h1ext.
  - Phase R: reasoner MLP (context -> expl -> recon), expl output written.
  - Phase 1: per window group, indirect-DMA gather h1ext[src] into
    [dst, j, 132]; attention exp-weights via small DVE/ACT ops; weighted
    feature sums via a free-dim reduce (no scatter needed); BN+ELU in
    feature-major after a PE transpose; project to h2ext; AllGather #2.
  - Phase 2: same gather structure on h2ext (34-wide), combine with recon,
    classifier matmul, log-softmax, outputs.

Self-contained: only needs numpy / ml_dtypes / concourse (bass) at runtime.
"""

import math
import os

import numpy as np
import ml_dtypes

import concourse.bass as bass
import concourse.mybir as mybir
import concourse.tile as tile
from concourse import bacc
from concourse.bass import IndirectOffsetOnAxis
from concourse.bass_utils import run_bass_kernel_spmd
from concourse.masks import make_identity
from concourse.tile_rust import add_dep_helper

BF16 = ml_dtypes.bfloat16
NCORES = 8
P = 128
GRP_W = 4  # windows per processing group
NEG_SLOPE = 0.2
RECON_W = 0.1
BN_EPS = 1e-5

F_IN = 128
HEADS = 4
C_HID = 32
HC = HEADS * C_HID  # 128
HID = 32
CTX_D = 6
EXPL_D = 32
N_CLS = 10
H1E = HC + HEADS  # gathered row: h1 (128) + alpha_src (4)
H1W = H1E + HEADS  # phase-0 psum row: + alpha_dst (4)
H2E = HID + 2  # h2 (32) + alpha_src2 + alpha_dst2

fp32 = mybir.dt.float32
bf16 = mybir.dt.bfloat16
i32 = mybir.dt.int32


# --------------------------------------------------------------------------
# Host-side graph preparation
# --------------------------------------------------------------------------
class Plan:
    pass


def build_plan(x, edge_index, context, W1, a_src1, a_dst1, b1,
               bn_gamma, bn_beta, bn_mean, bn_var,
               W2, a_src2, a_dst2, b2, rw1, rb1, rw2, rb2, dw, db, cw, cb):
    pl = Plan()
    N = x.shape[0]
    assert x.shape[1] == F_IN

    loop = np.arange(N, dtype=np.int64)
    src = np.concatenate([edge_index[0].astype(np.int64), loop])
    dst = np.concatenate([edge_index[1].astype(np.int64), loop])

    deg = np.bincount(dst, minlength=N)  # >= 1 (self loop)

    order = np.argsort(deg, kind="stable")  # ascending in-degree
    rank = np.empty(N, dtype=np.int64)
    rank[order] = np.arange(N)
    node_core = rank % NCORES
    node_slot = rank // NCORES  # local slot in [0, ceil(N/8))

    nloc_real = (N + NCORES - 1) // NCORES
    NLOC = ((nloc_real + 512 - 1) // 512) * 512  # pad to group multiple
    W = NLOC // P
    NGRP = W // GRP_W
    g_row = node_core * NLOC + node_slot  # row in the AllGather'd layout

    # per-window max degree (over all cores), uniform within each group
    win_of_node = node_slot // P
    nj_win = np.zeros(W, dtype=np.int64)
    np.maximum.at(nj_win, win_of_node, deg)
    nj_win = np.maximum(nj_win, 1)
    nj_grp = nj_win.reshape(NGRP, GRP_W).max(axis=1)
    nj_win_u = np.repeat(nj_grp, GRP_W)
    col0 = np.concatenate([[0], np.cumsum(nj_win_u)])
    T = int(col0[-1])

    # j-position of each edge within its destination's list
    sort_by_dst = np.argsort(dst, kind="stable")
    counts = deg
    starts = np.concatenate([[0], np.cumsum(counts)])[:-1]
    jpos = np.empty(len(dst), dtype=np.int64)
    jpos[sort_by_dst] = np.arange(len(dst)) - starts[dst[sort_by_dst]]

    e_core = node_core[dst]
    e_part = node_slot[dst] % P
    e_win = win_of_node[dst]
    e_col = col0[e_win] + jpos

    idx_all = np.zeros((NCORES, P, T), dtype=np.int32)
    mask_all = np.zeros((NCORES, P, T), dtype=np.float32)
    idx_all[e_core, e_part, e_col] = g_row[src].astype(np.int32)
    mask_all[e_core, e_part, e_col] = 1.0
    # pad destinations (slots with no real node): mask=1 so Z stays finite
    nreal = np.array([(N - k + NCORES - 1) // NCORES for k in range(NCORES)])
    for k in range(NCORES):
        for s in range(int(nreal[k]), NLOC):
            w_, p_ = s // P, s % P
            mask_all[k, p_, col0[w_]:col0[w_ + 1]] = 1.0
    pl.nreal = nreal

    # per-core x / context, padded + transposed
    xT = np.zeros((NCORES, F_IN, NLOC), dtype=BF16)
    ctxT = np.zeros((NCORES, CTX_D, NLOC), dtype=np.float32)
    perm = np.empty((NCORES, nloc_real), dtype=np.int64)
    for k in range(NCORES):
        nodes_k = order[k::NCORES]
        perm[k, :len(nodes_k)] = nodes_k
        xT[k, :, :len(nodes_k)] = x[nodes_k].T.astype(BF16)
        ctxT[k, :, :len(nodes_k)] = context[nodes_k].T

    # weight folding (layout prep only)
    wa_src1 = np.einsum("fhc,hc->fh", W1.reshape(F_IN, HEADS, C_HID), a_src1)
    wa_dst1 = np.einsum("fhc,hc->fh", W1.reshape(F_IN, HEADS, C_HID), a_dst1)
    W1e = np.concatenate([W1, wa_src1, wa_dst1], axis=1).astype(BF16)  # [128,136]
    wa_s2 = (W2 @ a_src2[0])[:, None]
    wa_d2 = (W2 @ a_dst2[0])[:, None]
    W2e = np.concatenate([W2, wa_s2, wa_d2], axis=1).astype(BF16)  # [128,34]
    bn_s = (bn_gamma / np.sqrt(bn_var + BN_EPS)).astype(np.float32)[:, None]
    bn_t = ((b1 - bn_mean) * bn_s[:, 0] + bn_beta).astype(np.float32)[:, None]

    pl.N, pl.NLOC, pl.W, pl.NGRP, pl.T = N, NLOC, W, NGRP, T
    pl.nloc_real = nloc_real
    pl.nj_grp = [int(v) for v in nj_grp]
    pl.col0 = [int(v) for v in col0]
    pl.perm = perm
    pl.in_maps = []
    shared = {
        "W1e": W1e,
        "W2e": W2e,
        "bn_s": bn_s, "bn_t": bn_t,
        "rw1": rw1.astype(np.float32),
        "rb1": rb1.astype(np.float32)[:, None],
        "rw2a": np.concatenate([rw2, rb2[None, :]], axis=0).astype(np.float32),
        "dwa": np.concatenate([dw * RECON_W, (db * RECON_W)[None, :]],
                              axis=0).astype(np.float32),
        "cwa": np.concatenate([cw, cb[None, :]], axis=0).astype(np.float32),
        "b2": b2.astype(np.float32)[:, None],
    }
    for k in range(NCORES):
        m = dict(shared)
        m["xT"] = xT[k]
        m["ctxT"] = ctxT[k]
        m["idx"] = idx_all[k]
        m["mask"] = mask_all[k].astype(BF16)
        pl.in_maps.append(m)
    return pl


# --------------------------------------------------------------------------
# Device program
# --------------------------------------------------------------------------
def build_program(pl, num_devices=NCORES):
    NLOC, W, NGRP = pl.NLOC, pl.W, pl.NGRP
    NALL = NCORES * NLOC
    nc = bacc.Bacc("TRN2", target_bir_lowering=False, debug=False,
                   num_devices=num_devices)

    din = {}
    for name, shape, dt in [
        ("xT", [F_IN, NLOC], bf16),
        ("ctxT", [CTX_D, NLOC], fp32),
        ("idx", [P, pl.T], i32),
        ("mask", [P, pl.T], bf16),
        ("W1e", [F_IN, H1W], bf16),
        ("W2e", [HC, H2E], bf16),
        ("bn_s", [HC, 1], fp32),
        ("bn_t", [HC, 1], fp32),
        ("rw1", [CTX_D, HID], fp32),
        ("rb1", [HID, 1], fp32),
        ("rw2a", [HID + 1, EXPL_D], fp32),
        ("dwa", [EXPL_D + 1, HID], fp32),
        ("cwa", [HID + 1, N_CLS], fp32),
        ("b2", [HID, 1], fp32),
    ]:
        din[name] = nc.dram_tensor(name, shape, dt, kind="ExternalInput").ap()
    lsm_out = nc.dram_tensor("lsm", [NLOC, N_CLS], fp32, kind="ExternalOutput").ap()
    expl_out = nc.dram_tensor("expl", [NLOC, EXPL_D], fp32, kind="ExternalOutput").ap()

    debug = bool(int(os.environ.get("GAT_DEBUG_DUMP", "0")))
    dbg = {}
    if debug:
        nj0 = pl.nj_grp[0]
        for name, shape, dt in [
            ("dbg_h1loc", [NLOC, H1E], bf16),
            ("dbg_h1full", [NALL, H1E], bf16),
            ("dbg_G0", [P, GRP_W * nj0 * H1E], bf16),
            ("dbg_expv0", [P, GRP_W * nj0 * HEADS], bf16),
            ("dbg_Wsum0", [P, GRP_W * HC], fp32),
            ("dbg_Zs0", [P, GRP_W * HEADS], fp32),
            ("dbg_y0", [P, GRP_W * P], fp32),
            ("dbg_hT0", [P, GRP_W * P], bf16),
            ("dbg_h2full", [NALL, H2E - 1], bf16),
            ("dbg_o2_0", [P, GRP_W * HID], fp32),
            ("dbg_pL0", [P, GRP_W * N_CLS], fp32),
        ]:
            dbg[name] = nc.dram_tensor(name, shape, dt, kind="ExternalOutput").ap()

    rg = [list(range(num_devices))]

    with tile.TileContext(nc) as tc:
        with (
            tc.tile_pool(name="const", bufs=1) as cpool,
            tc.tile_pool(name="dram", bufs=1, space="DRAM") as dpool,
        ):
            # persistent SBUF tensors
            sb = {}
            for name in ("W1e", "W2e", "bn_s", "bn_t", "rw1", "rb1", "rw2a",
                         "dwa", "cwa", "b2", "idx", "mask"):
                t = cpool.tile(list(din[name].shape), din[name].dtype, name=f"sb_{name}")
                nc.sync.dma_start(t[:], din[name][:])
                sb[name] = t
            ident = cpool.tile([P, P], fp32, name="ident")
            make_identity(nc, ident[:])
            ad1 = cpool.tile([P, W * HEADS], fp32, name="ad1")
            ad2 = cpool.tile([P, W], fp32, name="ad2")

            h1loc = dpool.tile([NLOC, H1E], bf16, name="h1loc")
            h1full = dpool.tile([NALL, H1E], bf16, addr_space="Shared", name="h1full")
            h2loc = dpool.tile([NLOC, H2E - 1], bf16, name="h2loc")
            h2full = dpool.tile([NALL, H2E - 1], bf16, addr_space="Shared", name="h2full")
            reconT_d = dpool.tile([EXPL_D, NLOC], fp32, name="reconT_d")

            # ---------------- Phase 0: h1ext ----------------
            with (
                tc.tile_pool(name="p0sb", bufs=3) as sp,
                tc.tile_pool(name="p0ps", bufs=2, space="PSUM") as pp,
            ):
                for w in range(W):
                    xq = sp.tile([P, P], bf16, name="xq")
                    nc.sync.dma_start(xq[:], din["xT"][:, w * P:(w + 1) * P])
                    p0 = pp.tile([P, H1W], fp32, name="p0")
                    nc.tensor.matmul(p0[:], lhsT=xq[:], rhs=sb["W1e"][:],
                                     start=True, stop=True)
                    h1s = sp.tile([P, H1E], bf16, name="h1s")
                    nc.scalar.activation(h1s[:], p0[:, 0:H1E],
                                         mybir.ActivationFunctionType.Copy)
                    nc.vector.tensor_copy(ad1[:, w * HEADS:(w + 1) * HEADS],
                                          p0[:, H1E:H1W])
                    nc.sync.dma_start(h1loc[w * P:(w + 1) * P, :], h1s[:])

            nc.gpsimd.collective_compute(
                "AllGather", mybir.AluOpType.bypass, replica_groups=rg,
                ins=[h1loc[:].opt()], outs=[h1full[:].opt()])
            if debug:
                nc.sync.dma_start(dbg["dbg_h1loc"][:], h1loc[:])
                nc.sync.dma_start(dbg["dbg_h1full"][:], h1full[:])

            # ---------------- Phase R: reasoner MLP ----------------
            with (
                tc.tile_pool(name="prsb", bufs=3) as sp,
                tc.tile_pool(name="prps", bufs=2, space="PSUM") as pp,
                tc.tile_pool(name="prpt", bufs=2, space="PSUM") as pt,
            ):
                for g in range(NGRP):
                    nsl = slice(g * 512, (g + 1) * 512)
                    cq = sp.tile([CTX_D, 512], fp32, name="cq")
                    nc.sync.dma_start(cq[:], din["ctxT"][:, nsl])
                    pA = pp.tile([HID, 512], fp32, name="pA")
                    nc.tensor.matmul(pA[:], lhsT=sb["rw1"][:], rhs=cq[:],
                                     start=True, stop=True)
                    A_s = sp.tile([HID + 1, 512], fp32, name="A_s")
                    nc.scalar.activation(A_s[0:HID, :], pA[:],
                                         mybir.ActivationFunctionType.Relu,
                                         bias=sb["rb1"][:, 0:1])
                    nc.vector.memset(A_s[HID:HID + 1, :], 1.0)
                    pE = pp.tile([EXPL_D, 512], fp32, name="pE")
                    nc.tensor.matmul(pE[:], lhsT=sb["rw2a"][:], rhs=A_s[:],
                                     start=True, stop=True)
                    eT = sp.tile([EXPL_D + 1, 512], fp32, name="eT")
                    nc.vector.tensor_copy(eT[0:EXPL_D, :], pE[:])
                    nc.vector.memset(eT[EXPL_D:EXPL_D + 1, :], 1.0)
                    pR = pp.tile([HID, 512], fp32, name="pR")
                    nc.tensor.matmul(pR[:], lhsT=sb["dwa"][:], rhs=eT[:],
                                     start=True, stop=True)
                    rT = sp.tile([HID, 512], fp32, name="rT")
                    nc.vector.tensor_copy(rT[:], pR[:])
                    nc.sync.dma_start(reconT_d[:, nsl], rT[:])
                    for i in range(GRP_W):
                        pT = pt.tile([P, EXPL_D], fp32, name="pT")
                        nc.tensor.transpose(
                            pT[:], eT[0:EXPL_D, i * P:(i + 1) * P],
                            ident[0:EXPL_D, 0:EXPL_D])
                        ex = sp.tile([P, EXPL_D], fp32, name="ex")
                        nc.vector.tensor_copy(ex[:], pT[:])
                        nc.sync.dma_start(
                            expl_out[(g * GRP_W + i) * P:(g * GRP_W + i + 1) * P, :],
                            ex[:])

            # ---------------- Phase 1: GAT layer 1 + h2ext ----------------
            with (
                tc.tile_pool(name="g1", bufs=2) as gp,
                tc.tile_pool(name="e1", bufs=2) as ep,
                tc.tile_pool(name="w1s", bufs=2) as wp,
                tc.tile_pool(name="x1", bufs=2) as xp,
                tc.tile_pool(name="p1t", bufs=2, space="PSUM") as ptp,
                tc.tile_pool(name="p1m", bufs=2, space="PSUM") as pmp,
            ):
                for g in range(NGRP):
                    nj = pl.nj_grp[g]
                    c0 = pl.col0[g * GRP_W]
                    ncol = GRP_W * nj
                    G = gp.tile([P, ncol * H1E], bf16, name="G", tag="G")
                    G3 = G[:].rearrange("p (t e) -> p t e", e=H1E)
                    fen = gp.tile([P, 8], bf16, name="fen", tag="fen")
                    for cc in range(0, ncol, 48):
                        ch = min(48, ncol - cc)
                        g_inst = nc.gpsimd.indirect_dma_start(
                            out=G3[:, cc:cc + ch, :], out_offset=None,
                            in_=h1full[:],
                            in_offset=IndirectOffsetOnAxis(
                                ap=sb["idx"][:, c0 + cc:c0 + cc + ch], axis=0))
                    f_inst = nc.gpsimd.dma_start(fen[:], h1full[0:P, 0:8])
                    add_dep_helper(f_inst.ins, g_inst.ins, True, "gfence")
                    G4h = G[:].rearrange("p (w t e) -> p w t e", w=GRP_W, e=H1E)

                    epre = ep.tile([P, ncol * HEADS], fp32, name="epre", tag="epre")
                    epre4 = epre[:].rearrange("p (w t h) -> p w t h",
                                              w=GRP_W, h=HEADS)
                    adw = ad1[:, g * GRP_W * HEADS:(g + 1) * GRP_W * HEADS]
                    ep_inst = nc.vector.tensor_tensor(
                        out=epre4, in0=G4h[:, :, :, HC:H1E],
                        in1=adw.rearrange("p (w h) -> p w h", h=HEADS)
                            .unsqueeze(2).to_broadcast([P, GRP_W, nj, HEADS]),
                        op=mybir.AluOpType.add)
                    add_dep_helper(ep_inst.ins, f_inst.ins, True, "gfence-r")
                    esc = ep.tile([P, ncol * HEADS], fp32, name="esc", tag="esc")
                    nc.vector.tensor_scalar_mul(esc[:], epre[:], NEG_SLOPE)
                    elr = ep.tile([P, ncol * HEADS], fp32, name="elr", tag="elr")
                    nc.vector.tensor_tensor(out=elr[:], in0=epre[:], in1=esc[:],
                                            op=mybir.AluOpType.max)
                    expf = ep.tile([P, ncol * HEADS], fp32, name="expf", tag="expf")
                    nc.scalar.activation(expf[:], elr[:],
                                         mybir.ActivationFunctionType.Exp)
                    expv = ep.tile([P, ncol * HEADS], bf16, name="expv", tag="expv")
                    nc.vector.tensor_tensor(
                        out=expv[:].rearrange("p (w t h) -> p w t h",
                                              w=GRP_W, h=HEADS),
                        in0=expf[:].rearrange("p (w t h) -> p w t h",
                                              w=GRP_W, h=HEADS),
                        in1=sb["mask"][:, c0:c0 + ncol]
                            .rearrange("p (w t) -> p w t", w=GRP_W)
                            .unsqueeze(3).to_broadcast([P, GRP_W, nj, HEADS]),
                        op=mybir.AluOpType.mult)
                    expA = wp.tile([P, ncol * HC], bf16, name="expA", tag="expA",
                                   bufs=1)
                    nc.scalar.activation(
                        expA[:].rearrange("p (w t h c) -> p w t h c",
                                          w=GRP_W, h=HEADS, c=C_HID),
                        expv[:].rearrange("p (w t h) -> p w t h",
                                          w=GRP_W, h=HEADS)
                            .unsqueeze(4).to_broadcast([P, GRP_W, nj, HEADS, C_HID]),
                        mybir.ActivationFunctionType.Copy)
                    Hw = wp.tile([P, ncol * HC], bf16, name="Hw", tag="Hw", bufs=1)
                    hw_inst = nc.vector.tensor_tensor(
                        out=Hw[:].rearrange("p (t c) -> p t c", c=HC),
                        in0=G3[:, :, 0:HC],
                        in1=expA[:].rearrange("p (t c) -> p t c", c=HC),
                        op=mybir.AluOpType.mult)
                    add_dep_helper(hw_inst.ins, f_inst.ins, True, "gfence-r2")
                    Wsum = xp.tile([P, GRP_W * HC], fp32, name="Wsum", tag="Wsum")
                    cur, buf = nj, Hw
                    lvl = 0
                    while cur > 1:
                        if cur % 2 == 1:
                            bv = buf[:].rearrange("p (w t c) -> p w t c",
                                                  w=GRP_W, c=HC)
                            nc.vector.tensor_tensor(
                                out=bv[:, :, 0:1, :], in0=bv[:, :, 0:1, :],
                                in1=bv[:, :, cur - 1:cur, :],
                                op=mybir.AluOpType.add)
                            cur -= 1
                        half = cur // 2
                        dt_out = bf16 if half > 1 else fp32
                        nxt = (wp.tile([P, GRP_W * half * HC], dt_out,
                                       name=f"tr{lvl}", tag="tr0" if lvl % 2 == 0
                                       else "tr1", bufs=1)
                               if half > 1 else Wsum)
                        bv = (buf[:].rearrange("p (w tc) -> p w tc", w=GRP_W)
                              [:, :, 0:cur * HC]
                              .rearrange("p w (t2 pc) -> p w t2 pc", pc=2 * HC))
                        nc.vector.tensor_tensor(
                            out=nxt[:].rearrange("p (w t c) -> p w t c",
                                                 w=GRP_W, c=HC),
                            in0=bv[:, :, :, 0:HC], in1=bv[:, :, :, HC:2 * HC],
                            op=mybir.AluOpType.add)
                        cur, buf, lvl = half, nxt, lvl + 1
                    if nj == 1:
                        nc.vector.tensor_copy(Wsum[:], Hw[:])
                    Zs = xp.tile([P, GRP_W * HEADS], fp32, name="Zs", tag="Zs")
                    nc.vector.tensor_reduce(
                        out=Zs[:].rearrange("p (w h) -> p w h", w=GRP_W),
                        in_=expv[:].rearrange("p (w t h) -> p w h t",
                                              w=GRP_W, h=HEADS),
                        axis=mybir.AxisListType.X, op=mybir.AluOpType.add)
                    rZ = xp.tile([P, GRP_W * HEADS], fp32, name="rZ", tag="rZ")
                    nc.vector.reciprocal(rZ[:], Zs[:])
                    u = xp.tile([P, GRP_W * HC], fp32, name="u", tag="u")
                    nc.vector.tensor_tensor(
                        out=u[:].rearrange("p (w h c) -> p w h c",
                                           h=HEADS, c=C_HID),
                        in0=Wsum[:].rearrange("p (w h c) -> p w h c",
                                              h=HEADS, c=C_HID),
                        in1=rZ[:].rearrange("p (w h) -> p w h", h=HEADS)
                            .unsqueeze(3).to_broadcast([P, GRP_W, HEADS, C_HID]),
                        op=mybir.AluOpType.mult)
                    uT = ptp.tile([P, GRP_W * P], fp32, name="uT", tag="uT")
                    for i in range(GRP_W):
                        nc.tensor.transpose(uT[:, i * P:(i + 1) * P],
                                            u[:, i * P:(i + 1) * P], ident[:])
                    y = xp.tile([P, GRP_W * P], fp32, name="y", tag="y")
                    nc.scalar.activation(y[:], uT[:],
                                         mybir.ActivationFunctionType.Identity,
                                         bias=sb["bn_t"][:, 0:1],
                                         scale=sb["bn_s"][:, 0:1])
                    neg = xp.tile([P, GRP_W * P], fp32, name="neg", tag="neg")
                    nc.vector.tensor_scalar_min(neg[:], y[:], 0.0)
                    eneg = xp.tile([P, GRP_W * P], fp32, name="eneg", tag="eneg")
                    nc.scalar.activation(eneg[:], neg[:],
                                         mybir.ActivationFunctionType.Exp)
                    rel = xp.tile([P, GRP_W * P], fp32, name="rel", tag="rel")
                    nc.vector.tensor_scalar_max(rel[:], y[:], 0.0)
                    s1 = xp.tile([P, GRP_W * P], fp32, name="s1", tag="s1")
                    nc.vector.tensor_tensor(out=s1[:], in0=rel[:], in1=eneg[:],
                                            op=mybir.AluOpType.add)
                    hT = xp.tile([P, GRP_W * P], bf16, name="hT", tag="hT")
                    nc.vector.tensor_scalar(out=hT[:], in0=s1[:], scalar1=1.0,
                                            scalar2=None,
                                            op0=mybir.AluOpType.subtract)
                    if debug and g == 0:
                        nc.sync.dma_start(dbg["dbg_G0"][:], G[:])
                        nc.sync.dma_start(dbg["dbg_expv0"][:], expv[:])
                        nc.sync.dma_start(dbg["dbg_Wsum0"][:], Wsum[:])
                        nc.sync.dma_start(dbg["dbg_Zs0"][:], Zs[:])
                        nc.sync.dma_start(dbg["dbg_y0"][:], y[:])
                        nc.sync.dma_start(dbg["dbg_hT0"][:], hT[:])
                    for i in range(GRP_W):
                        w = g * GRP_W + i
                        p2 = pmp.tile([P, H2E], fp32, name="p2", tag="p2")
                        nc.tensor.matmul(p2[:], lhsT=hT[:, i * P:(i + 1) * P],
                                         rhs=sb["W2e"][:], start=True, stop=True)
                        h2s = xp.tile([P, H2E - 1], bf16, name="h2s", tag="h2s")
                        nc.vector.tensor_copy(h2s[:], p2[:, 0:H2E - 1])
                        nc.vector.tensor_copy(ad2[:, w:w + 1], p2[:, H2E - 1:H2E])
                        nc.sync.dma_start(h2loc[w * P:(w + 1) * P, :], h2s[:])

            nc.gpsimd.collective_compute(
                "AllGather", mybir.AluOpType.bypass, replica_groups=rg,
                ins=[h2loc[:].opt()], outs=[h2full[:].opt()])
            if debug:
                nc.sync.dma_start(dbg["dbg_h2full"][:], h2full[:])

            # ---------------- Phase 2: GAT layer 2 + head ----------------
            H2R = H2E - 1  # 33 cols in gathered rows
            with (
                tc.tile_pool(name="g2", bufs=2) as gp,
                tc.tile_pool(name="e2", bufs=2) as ep,
                tc.tile_pool(name="x2", bufs=2) as xp,
                tc.tile_pool(name="p2t", bufs=2, space="PSUM") as ptp,
                tc.tile_pool(name="p2l", bufs=2, space="PSUM") as plp,
            ):
                for g in range(NGRP):
                    nj = pl.nj_grp[g]
                    c0 = pl.col0[g * GRP_W]
                    ncol = GRP_W * nj
                    G2 = gp.tile([P, ncol * H2R], bf16, name="G2", tag="G2")
                    G23 = G2[:].rearrange("p (t e) -> p t e", e=H2R)
                    fen2 = gp.tile([P, 8], bf16, name="fen2", tag="fen2")
                    for cc in range(0, ncol, 48):
                        ch = min(48, ncol - cc)
                        g2_inst = nc.gpsimd.indirect_dma_start(
                            out=G23[:, cc:cc + ch, :], out_offset=None,
                            in_=h2full[:],
                            in_offset=IndirectOffsetOnAxis(
                                ap=sb["idx"][:, c0 + cc:c0 + cc + ch], axis=0))
                    f2_inst = nc.gpsimd.dma_start(fen2[:], h2full[0:P, 0:8])
                    add_dep_helper(f2_inst.ins, g2_inst.ins, True, "g2fence")
                    ep2 = ep.tile([P, ncol], fp32, name="ep2", tag="ep2")
                    e2_inst = nc.vector.tensor_tensor(
                        out=ep2[:].rearrange("p (w t) -> p w t", w=GRP_W),
                        in0=G23[:, :, HID:HID + 1]
                            .rearrange("p t one -> p (t one)")
                            .rearrange("p (w t) -> p w t", w=GRP_W),
                        in1=ad2[:, g * GRP_W:(g + 1) * GRP_W]
                            .unsqueeze(2).to_broadcast([P, GRP_W, nj]),
                        op=mybir.AluOpType.add)
                    add_dep_helper(e2_inst.ins, f2_inst.ins, True, "g2fence-r")
                    es2 = ep.tile([P, ncol], fp32, name="es2", tag="es2")
                    nc.vector.tensor_scalar_mul(es2[:], ep2[:], NEG_SLOPE)
                    el2 = ep.tile([P, ncol], fp32, name="el2", tag="el2")
                    nc.vector.tensor_tensor(out=el2[:], in0=ep2[:], in1=es2[:],
                                            op=mybir.AluOpType.max)
                    ex2f = ep.tile([P, ncol], fp32, name="ex2f", tag="ex2f")
                    nc.scalar.activation(ex2f[:], el2[:],
                                         mybir.ActivationFunctionType.Exp)
                    ex2 = ep.tile([P, ncol], bf16, name="ex2", tag="ex2")
                    nc.vector.tensor_tensor(out=ex2[:], in0=ex2f[:],
                                            in1=sb["mask"][:, c0:c0 + ncol],
                                            op=mybir.AluOpType.mult)
                    exA2 = ep.tile([P, ncol * HID], bf16, name="exA2", tag="exA2")
                    nc.scalar.activation(
                        exA2[:].rearrange("p (t c) -> p t c", c=HID),
                        ex2[:].unsqueeze(2).to_broadcast([P, ncol, HID]),
                        mybir.ActivationFunctionType.Copy)
                    Hw2 = ep.tile([P, ncol * HID], bf16, name="Hw2", tag="Hw2")
                    hw2_inst = nc.vector.tensor_tensor(
                        out=Hw2[:].rearrange("p (t c) -> p t c", c=HID),
                        in0=G23[:, :, 0:HID],
                        in1=exA2[:].rearrange("p (t c) -> p t c", c=HID),
                        op=mybir.AluOpType.mult)
                    add_dep_helper(hw2_inst.ins, f2_inst.ins, True, "g2fence-r2")
                    Ws2 = xp.tile([P, GRP_W * HID], fp32, name="Ws2", tag="Ws2")
                    nc.vector.tensor_reduce(
                        out=Ws2[:].rearrange("p (w c) -> p w c", w=GRP_W),
                        in_=Hw2[:].rearrange("p (w t c) -> p w c t",
                                             w=GRP_W, c=HID),
                        axis=mybir.AxisListType.X, op=mybir.AluOpType.add)
                    Z2 = xp.tile([P, GRP_W], fp32, name="Z2", tag="Z2")
                    nc.vector.tensor_reduce(
                        out=Z2[:], in_=ex2[:].rearrange("p (w t) -> p w t",
                                                        w=GRP_W),
                        axis=mybir.AxisListType.X, op=mybir.AluOpType.add)
                    rZ2 = xp.tile([P, GRP_W], fp32, name="rZ2", tag="rZ2")
                    nc.vector.reciprocal(rZ2[:], Z2[:])
                    o2 = xp.tile([P, GRP_W * HID], fp32, name="o2", tag="o2")
                    nc.vector.tensor_tensor(
                        out=o2[:].rearrange("p (w c) -> p w c", w=GRP_W),
                        in0=Ws2[:].rearrange("p (w c) -> p w c", w=GRP_W),
                        in1=rZ2[:].unsqueeze(2).to_broadcast([P, GRP_W, HID]),
                        op=mybir.AluOpType.mult)
                    o2T = ptp.tile([HID, GRP_W * P], fp32, name="o2T", tag="o2T")
                    for i in range(GRP_W):
                        nc.tensor.transpose(o2T[:, i * P:(i + 1) * P],
                                            o2[:, i * HID:(i + 1) * HID],
                                            ident[:])
                    tmpb = xp.tile([HID, GRP_W * P], fp32, name="tmpb", tag="tmpb")
                    nc.scalar.activation(tmpb[:], o2T[:],
                                         mybir.ActivationFunctionType.Identity,
                                         bias=sb["b2"][:, 0:1])
                    rT2 = xp.tile([HID, GRP_W * P], fp32, name="rT2", tag="rT2")
                    nc.sync.dma_start(rT2[:],
                                      reconT_d[:, g * 512:(g + 1) * 512])
                    comb = xp.tile([HID + 1, GRP_W * P], fp32, name="comb",
                                   tag="comb")
                    nc.vector.tensor_tensor(out=comb[0:HID, :], in0=tmpb[:],
                                            in1=rT2[:], op=mybir.AluOpType.add)
                    nc.vector.memset(comb[HID:HID + 1, :], 1.0)
                    pL = plp.tile([P, GRP_W * N_CLS], fp32, name="pL", tag="pL")
                    for i in range(GRP_W):
                        nc.tensor.matmul(pL[:, i * N_CLS:(i + 1) * N_CLS],
                                         lhsT=comb[:, i * P:(i + 1) * P],
                                         rhs=sb["cwa"][:], start=True, stop=True)
                    if debug and g == 0:
                        nc.sync.dma_start(dbg["dbg_o2_0"][:], o2[:])
                        pLc = xp.tile([P, GRP_W * N_CLS], fp32, name="pLc",
                                      tag="pLc")
                        nc.vector.tensor_copy(pLc[:], pL[:])
                        nc.sync.dma_start(dbg["dbg_pL0"][:], pLc[:])
                    mx = xp.tile([P, GRP_W], fp32, name="mx", tag="mx")
                    nc.vector.tensor_reduce(
                        out=mx[:], in_=pL[:].rearrange("p (w c) -> p w c",
                                                       w=GRP_W),
                        axis=mybir.AxisListType.X, op=mybir.AluOpType.max)
                    um = xp.tile([P, GRP_W * N_CLS], fp32, name="um", tag="um")
                    nc.vector.tensor_tensor(
                        out=um[:].rearrange("p (w c) -> p w c", w=GRP_W),
                        in0=pL[:].rearrange("p (w c) -> p w c", w=GRP_W),
                        in1=mx[:].unsqueeze(2).to_broadcast([P, GRP_W, N_CLS]),
                        op=mybir.AluOpType.subtract)
                    pe = xp.tile([P, GRP_W * N_CLS], fp32, name="pe", tag="pe")
                    nc.scalar.activation(pe[:], um[:],
                                         mybir.ActivationFunctionType.Exp)
                    ssum = xp.tile([P, GRP_W], fp32, name="ssum", tag="ssum")
                    nc.vector.tensor_reduce(
                        out=ssum[:], in_=pe[:].rearrange("p (w c) -> p w c",
                                                         w=GRP_W),
                        axis=mybir.AxisListType.X, op=mybir.AluOpType.add)
                    lss = xp.tile([P, GRP_W], fp32, name="lss", tag="lss")
                    nc.scalar.activation(lss[:], ssum[:],
                                         mybir.ActivationFunctionType.Ln)
                    lsm = xp.tile([P, GRP_W * N_CLS], fp32, name="lsmt", tag="lsmt")
                    nc.vector.tensor_tensor(
                        out=lsm[:].rearrange("p (w c) -> p w c", w=GRP_W),
                        in0=um[:].rearrange("p (w c) -> p w c", w=GRP_W),
                        in1=lss[:].unsqueeze(2).to_broadcast([P, GRP_W, N_CLS]),
                        op=mybir.AluOpType.subtract)
                    for i in range(GRP_W):
                        w = g * GRP_W + i
                        nc.sync.dma_start(lsm_out[w * P:(w + 1) * P, :],
                                          lsm[:, i * N_CLS:(i + 1) * N_CLS])

    nc.compile()
    return nc


# --------------------------------------------------------------------------
# Entry point
# --------------------------------------------------------------------------
_LAST_RESULT = {}


def kernel(**inputs):
    inputs = {k: np.asarray(v) for k, v in inputs.items()}
    pl = build_plan(**inputs)
    nc = build_program(pl)
    trace = bool(int(os.environ.get("GAT_TRACE", "0")))
    res = run_bass_kernel_spmd(nc, pl.in_maps, core_ids=list(range(NCORES)),
                               trace=trace)
    _LAST_RESULT["res"] = res
    N = pl.N
    lsm = np.empty((N, N_CLS), dtype=np.float32)
    expl = np.empty((N, EXPL_D), dtype=np.float32)
    for k in range(NCORES):
        nreal = int(pl.nreal[k])
        nodes_k = pl.perm[k][:nreal]
        lsm[nodes_k] = res.results[k]["lsm"][:nreal]
        expl[nodes_k] = res.results[k]["expl"][:nreal]
    return lsm, expl
